# revision 10
# baseline (speedup 1.0000x reference)
"""Trainium2 Bass kernel for CSAttention.

Reference computation (per batch b of 32, N=1024 tokens, C=512 channels,
L=512 latent):
    qk  = x @ W_qk.T + b_qk            # [N, 2L]
    q   = qk[:, :L] * L**-0.5
    k   = qk[:, L:]
    out = softmax(q @ k.T, -1) @ y     # [N, C]

Sharding: data-parallel over the batch axis across 8 NeuronCores
(4 batches per core); W_qk / b_qk replicated.

Per-core kernel structure (all matmuls float32r = full PE rate, fp32 store):
  prep:    W  --PE-transpose-->  WT [C, 2L]   (once)
  stage A: x[b] --PE-transpose--> XT [C, N]   (per batch, double-buffered)
  stage B: QKT[l, n] = (WT col-slice).T @ XT          (+bias)   [2L, N]
  stage C: ST[m, n]  = (KT col-slice).T @ QT   -> exp(scale*.)  [N, N]
  stage D: out[n, :] = (ET col-slice).T @ [Y | 1]   ; the appended ones
           column produces the softmax denominator in the same PSUM
           accumulation; normalize with DVE reciprocal + per-partition mul.
"""

import numpy as np

import concourse.bass as bass
import concourse.mybir as mybir
import concourse.tile as tile
from concourse import bacc
from concourse.bass_utils import run_bass_kernel_spmd
from concourse.masks import make_identity

P = 128
N_CORES = 8
B_FULL = 32
B_PER_CORE = B_FULL // N_CORES  # 4
N = 1024            # tokens
C = 512             # channels
L = 512             # latent
TWO_L = 2 * L
NT = N // P         # 8 token tiles
CT = C // P         # 4 channel tiles
LT = TWO_L // P     # 8 latent tiles (0..3 = q, 4..7 = k)
SCALE = float(L) ** -0.5
YA = C + 2          # augmented Y width: [Y | ones | pad] (fp32r needs even dims)
NA = 258            # first matmul free dim (>=256 keeps fp32r at full rate)
NB = YA - NA        # 256
SCOL = C - NA       # ones column's index within psum_B (= 254)

F32 = mybir.dt.float32
F32R = mybir.dt.float32r
IDENT = mybir.ActivationFunctionType.Identity
EXP = mybir.ActivationFunctionType.Exp


def _r(ap):
    return ap.bitcast(F32R)


def _emit(tc, x, y, w, bvec, out):
    """Emit the per-core kernel. x/y: [B_PER_CORE, N, C] dram APs,
    w: [2L, C], bvec: [2L], out: [B_PER_CORE, N, C]."""
    from contextlib import ExitStack

    nc = tc.nc
    with ExitStack() as ctx:
        const = ctx.enter_context(tc.tile_pool(name="const", bufs=1))
        big = ctx.enter_context(tc.tile_pool(name="big", bufs=1))
        outp = ctx.enter_context(tc.tile_pool(name="outp", bufs=3))
        rsp = ctx.enter_context(tc.tile_pool(name="rsp", bufs=3))
        ps_mm = ctx.enter_context(tc.tile_pool(name="ps_mm", bufs=2, space="PSUM"))
        ps_d = ctx.enter_context(tc.tile_pool(name="ps_d", bufs=4, space="PSUM"))
        ps_tr = ctx.enter_context(tc.tile_pool(name="ps_tr", bufs=2, space="PSUM"))

        identity = const.tile([P, P], F32, tag="ident")
        make_identity(nc, identity)

        # b_qk striped so partition p, col t  <-  b_qk[t*128 + p]
        bias_sb = const.tile([P, LT], F32, tag="bias")
        nc.sync.dma_start(bias_sb, bvec.rearrange("(o p) -> p o", p=P))

        # ---- one-time: WT[c, l] = W[l, c] transposed via PE ----
        # All matmul-input tiles are float32r: the producing DVE/ACT op
        # rounds fp32 -> fp32r (1s8e11m, low 12 bits zero), which the BIR
        # verifier requires for fp32r matmul operands.
        wnat = big.tile([P, LT, C], F32, tag="wnat")
        nc.sync.dma_start(wnat, w.rearrange("(t p) c -> p t c", p=P))
        wt = big.tile([P, CT, TWO_L], F32R, tag="wt")
        for lt in range(LT):
            for ct in range(CT):
                ps = ps_tr.tile([P, P], F32, tag="tr")
                nc.tensor.transpose(ps, wnat[:, lt, ct * P:(ct + 1) * P], identity)
                nc.vector.tensor_copy(wt[:, ct, lt * P:(lt + 1) * P], ps)

        # ---- persistent per-batch workspaces ----
        xnat = big.tile([P, NT, C], F32, tag="xnat")
        xt2 = [
            big.tile([P, CT, N], F32R, tag=f"xt{j}", name=f"xt{j}") for j in range(2)
        ]
        qkt = big.tile([P, LT, N], F32R, tag="qkt")     # rows l, cols n
        et = big.tile([P, NT, N], F32R, tag="et")       # rows m, cols n
        ynat = big.tile([P, NT, YA], F32, tag="ynat")   # staging [Y | ones]
        yaug = big.tile([P, NT, YA], F32R, tag="yaug")  # [Y | ones]
        nc.vector.memset(ynat[:, :, C:YA], 1.0)

        def load_x(i):
            nc.sync.dma_start(xnat, x[i].rearrange("(t p) c -> p t c", p=P))

        def transpose_x(i):
            dst = xt2[i % 2]
            for nt_ in range(NT):
                for ct in range(CT):
                    ps = ps_tr.tile([P, P], F32, tag="tr")
                    nc.tensor.transpose(
                        ps, xnat[:, nt_, ct * P:(ct + 1) * P], identity
                    )
                    nc.vector.tensor_copy(dst[:, ct, nt_ * P:(nt_ + 1) * P], ps)

        def load_y(i):
            # DMA to fp32 staging, then one DVE copy rounds into fp32r yaug
            nc.sync.dma_start(ynat[:, :, 0:C], y[i].rearrange("(t p) c -> p t c", p=P))
            nc.vector.tensor_copy(yaug, ynat)

        load_x(0)
        transpose_x(0)

        for i in range(B_PER_CORE):
            load_y(i)
            xt = xt2[i % 2]

            # ---- stage B: QKT = WT.T @ XT (+ bias) ----
            for nh in range(2):
                for lt in range(LT):
                    ps = ps_mm.tile([P, 512], F32, tag="mm")
                    for ct in range(CT):
                        nc.tensor.matmul(
                            ps,
                            (wt[:, ct, lt * P:(lt + 1) * P]),
                            (xt[:, ct, nh * 512:(nh + 1) * 512]),
                            start=(ct == 0),
                            stop=(ct == CT - 1),
                        )
                    nc.scalar.activation(
                        qkt[:, lt, nh * 512:(nh + 1) * 512],
                        ps,
                        IDENT,
                        bias=bias_sb[:, lt:lt + 1],
                    )

            # prefetch + transpose next batch's x while B's results copy out
            if i + 1 < B_PER_CORE:
                load_x(i + 1)
                transpose_x(i + 1)

            # ---- stage C: ST[m, n] = K[m] . Q[n] ; ET = exp(scale * ST) ----
            for mt in range(NT):
                for nh in range(2):
                    ps = ps_mm.tile([P, 512], F32, tag="mm")
                    for lq in range(4):
                        nc.tensor.matmul(
                            ps,
                            (qkt[:, 4 + lq, mt * P:(mt + 1) * P]),
                            (qkt[:, lq, nh * 512:(nh + 1) * 512]),
                            start=(lq == 0),
                            stop=(lq == 3),
                        )
                    nc.scalar.activation(
                        et[:, mt, nh * 512:(nh + 1) * 512], ps, EXP, scale=SCALE
                    )

            # ---- stage D: out = ET.T @ [Y | 1], then normalize ----
            for nt_ in range(NT):
                psA = ps_d.tile([P, NA], F32, tag="d")
                psB = ps_d.tile([P, NA], F32, tag="d")
                for mt in range(NT):
                    lw = (et[:, mt, nt_ * P:(nt_ + 1) * P])
                    nc.tensor.matmul(
                        psA, lw, (yaug[:, mt, 0:NA]),
                        start=(mt == 0), stop=(mt == NT - 1),
                    )
                    nc.tensor.matmul(
                        psB[:, 0:NB], lw, (yaug[:, mt, NA:YA]),
                        start=(mt == 0), stop=(mt == NT - 1),
                    )
                rs = rsp.tile([P, 1], F32, tag="rs")
                nc.vector.reciprocal(rs, psB[:, SCOL:SCOL + 1])
                ob = outp.tile([P, C], F32, tag="ob")
                nc.vector.tensor_scalar_mul(ob[:, 0:NA], psA[:, 0:NA], rs)
                nc.vector.tensor_scalar_mul(ob[:, NA:C], psB[:, 0:SCOL], rs)
                nc.sync.dma_start(out[i, nt_ * P:(nt_ + 1) * P, :], ob)


_NC_CACHE = {}


def _build():
    if "nc" in _NC_CACHE:
        return _NC_CACHE["nc"]
    nc = bacc.Bacc(
        "TRN2",
        target_bir_lowering=False,
        debug=False,
        enable_asserts=False,
        num_devices=N_CORES,
    )
    x = nc.dram_tensor("x", [B_PER_CORE, N, C], F32, kind="ExternalInput").ap()
    y = nc.dram_tensor("y", [B_PER_CORE, N, C], F32, kind="ExternalInput").ap()
    w = nc.dram_tensor("W_qk", [TWO_L, C], F32, kind="ExternalInput").ap()
    bvec = nc.dram_tensor("b_qk", [TWO_L], F32, kind="ExternalInput").ap()
    out = nc.dram_tensor("out", [B_PER_CORE, N, C], F32, kind="ExternalOutput").ap()
    with tile.TileContext(nc) as tc:
        _emit(tc, x, y, w, bvec, out)
    nc.compile()
    _NC_CACHE["nc"] = nc
    return nc


def run(x, y, W_qk, b_qk, trace=False):
    """Run the SPMD kernel on 8 cores; returns (out, BassKernelResults)."""
    nc = _build()
    x = np.ascontiguousarray(x, dtype=np.float32)
    y = np.ascontiguousarray(y, dtype=np.float32)
    W_qk = np.ascontiguousarray(W_qk, dtype=np.float32)
    b_qk = np.ascontiguousarray(b_qk, dtype=np.float32)
    in_maps = [
        {
            "x": x[k * B_PER_CORE:(k + 1) * B_PER_CORE],
            "y": y[k * B_PER_CORE:(k + 1) * B_PER_CORE],
            "W_qk": W_qk,
            "b_qk": b_qk,
        }
        for k in range(N_CORES)
    ]
    res = run_bass_kernel_spmd(
        nc, in_maps, core_ids=list(range(N_CORES)), trace=trace
    )
    outs = [r["out"] for r in res.results]
    return np.concatenate(outs, axis=0), res


def kernel(x, y, W_qk, b_qk):
    out, _ = run(x, y, W_qk, b_qk)
    return out
